# revision 6
# baseline (speedup 1.0000x reference)
"""Trainium2 Bass kernel for nn_CriticTab (embedding-lookup critic table).

Reference math: out[b] = sum_n [obs[b,:] exactly matches bits of mask row n] * v[n].
Since the match indicator keys on the packed bit-pattern of each mask row, the
(mask, v) pair folds on the host into a 65536-entry table indexed by packed obs
bits (exact for ANY {-1,+1} mask, including permuted/duplicate/missing rows —
collisions sum, missing keys give 0, which is precisely the reference's
multi-/zero-hot `ind @ v`). The device kernel packs obs bits into an index
(DVE shift-by-iota + reduce-add) and gathers from the table in HBM via SWDGE
indirect DMA (one offset per partition per gather → 4 gathers of 128).
Data-parallel over B across the 8 cores.
"""

import numpy as np

B, D, N = 4096, 16, 65536
N_CORES = 8
BS = B // N_CORES  # 512 observations per core
P = 128            # SBUF partitions
G = BS // P        # 4 groups of 128 observations

_CACHE = {}

LAST_RESULT = None  # BassKernelResults of the most recent run (for test harness)


def _program():
    """Build + compile the single-core SPMD Bass program (cached per process)."""
    if "nc" in _CACHE:
        return _CACHE["nc"]

    import concourse.bacc as bacc
    import concourse.bass as bass
    import concourse.mybir as mybir
    import concourse.tile as tile

    nc = bacc.Bacc("TRN2", debug=False, enable_asserts=False, num_devices=N_CORES)
    obs_d = nc.dram_tensor("obs", [BS, D], mybir.dt.int32, kind="ExternalInput")
    v_d = nc.dram_tensor("v", [N, 1], mybir.dt.float32, kind="ExternalInput")
    out_d = nc.dram_tensor("out", [BS], mybir.dt.float32, kind="ExternalOutput")

    with tile.TileContext(nc) as tc:
        with tc.tile_pool(name="p", bufs=1) as pool:
            obs_t = pool.tile([P, G * D], mybir.dt.int32)
            sh_t = pool.tile([P, D], mybir.dt.int32)
            prod_t = pool.tile([P, D], mybir.dt.int32)
            idx_t = pool.tile([P, G], mybir.dt.int32)
            g_t = pool.tile([P, G], mybir.dt.float32)

            # obs rows p*G+g land on partition p, giving each partition one
            # contiguous 256B run of DRAM.
            nc.sync.dma_start(
                out=obs_t[:], in_=obs_d[:].rearrange("(p g) d -> p (g d)", p=P)
            )
            # per-element shift amounts 0..15 on every partition (runs during
            # the obs DMA latency window)
            nc.gpsimd.iota(sh_t[:], pattern=[[1, D]], channel_multiplier=0)

            with nc.allow_low_precision(reason="exact small-int bit packing"):
                for j in range(G):
                    # idx = sum_d obs[:,d] << d
                    nc.vector.tensor_tensor(
                        out=prod_t[:],
                        in0=obs_t[:, j * D : (j + 1) * D],
                        in1=sh_t[:],
                        op=mybir.AluOpType.logical_shift_left,
                    )
                    nc.vector.tensor_reduce(
                        out=idx_t[:, j : j + 1],
                        in_=prod_t[:],
                        axis=mybir.AxisListType.X,
                        op=mybir.AluOpType.add,
                    )
                    # HW indirect DMA: ONE offset per partition; fetches the
                    # out free-dim length from that offset. 128 rows per shot.
                    nc.gpsimd.indirect_dma_start(
                        out=g_t[:, j : j + 1],
                        out_offset=None,
                        in_=v_d[:],
                        in_offset=bass.IndirectOffsetOnAxis(
                            ap=idx_t[:, j : j + 1], axis=0
                        ),
                        oob_is_err=False,
                    )
                    # store each group as soon as its gather lands, so the
                    # kernel tail waits only on the last gather, not all four
                    nc.sync.dma_start(
                        out=out_d[:].rearrange("(p g) -> p g", p=P)[:, j : j + 1],
                        in_=g_t[:, j : j + 1],
                    )

    nc.compile()
    _CACHE["nc"] = nc
    return nc


def _fold_table(mask: np.ndarray, v: np.ndarray) -> np.ndarray:
    """Fold (mask, v) into table[key] = sum of v[n] over rows n with that key."""
    pw = 1 << np.arange(D, dtype=np.int64)
    m01 = (np.asarray(mask).astype(np.int64) + 1) // 2
    keys = (m01 * pw[None, :]).sum(axis=1)
    if np.array_equal(keys, np.arange(N, dtype=np.int64)):
        return v  # the actual setup: mask enumerates all states in order
    table = np.zeros(N, dtype=np.float32)
    np.add.at(table, keys, v)
    return table


def kernel(obs, mask, v):
    global LAST_RESULT
    from concourse.bass_utils import run_bass_kernel_spmd

    obs = np.ascontiguousarray(np.asarray(obs), dtype=np.int32)
    v = np.ascontiguousarray(np.asarray(v), dtype=np.float32)
    table = np.ascontiguousarray(_fold_table(mask, v)).reshape(N, 1)

    nc = _program()
    in_maps = [
        {"obs": obs[i * BS : (i + 1) * BS], "v": table} for i in range(N_CORES)
    ]
    res = run_bass_kernel_spmd(nc, in_maps, list(range(N_CORES)))
    LAST_RESULT = res
    return np.concatenate(
        [res.results[i]["out"].reshape(BS) for i in range(N_CORES)]
    )


# revision 7
# speedup vs baseline: 1.0942x; 1.0942x over previous
"""Trainium2 Bass kernel for nn_CriticTab (embedding-lookup critic table).

Reference math: out[b] = sum_n [obs[b,:] exactly matches bits of mask row n] * v[n].
Since the match indicator keys on the packed bit-pattern of each mask row, the
(mask, v) pair folds on the host into a 65536-entry table indexed by packed obs
bits (exact for ANY {-1,+1} mask, including permuted/duplicate/missing rows —
collisions sum, missing keys give 0, which is precisely the reference's
multi-/zero-hot `ind @ v`). The device kernel packs obs bits into an index
(DVE shift-by-iota + reduce-add) and gathers from the table in HBM via SWDGE
indirect DMA (one offset per partition per gather → 4 gathers of 128).
Data-parallel over B across the 8 cores.
"""

import numpy as np

B, D, N = 4096, 16, 65536
N_CORES = 8
BS = B // N_CORES  # 512 observations per core
P = 128            # SBUF partitions
G = BS // P        # 4 groups of 128 observations

_CACHE = {}

LAST_RESULT = None  # BassKernelResults of the most recent run (for test harness)


def _program():
    """Build + compile the single-core SPMD Bass program (cached per process)."""
    if "nc" in _CACHE:
        return _CACHE["nc"]

    import concourse.bacc as bacc
    import concourse.bass as bass
    import concourse.mybir as mybir
    import concourse.tile as tile

    nc = bacc.Bacc("TRN2", debug=False, enable_asserts=False, num_devices=N_CORES)
    obs_d = nc.dram_tensor("obs", [BS, D], mybir.dt.int32, kind="ExternalInput")
    v_d = nc.dram_tensor("v", [N, 1], mybir.dt.float32, kind="ExternalInput")
    out_d = nc.dram_tensor("out", [BS], mybir.dt.float32, kind="ExternalOutput")

    with tile.TileContext(nc) as tc:
        with tc.tile_pool(name="p", bufs=1) as pool:
            obs_t = pool.tile([P, G * D], mybir.dt.int32)
            sh_t = pool.tile([P, D], mybir.dt.int32)
            prod_t = pool.tile([P, D], mybir.dt.int32)
            idx_t = pool.tile([P, G], mybir.dt.int32)
            g_t = pool.tile([P, G], mybir.dt.float32)

            # obs rows p*G+g land on partition p, giving each partition one
            # contiguous 256B run of DRAM.
            nc.sync.dma_start(
                out=obs_t[:], in_=obs_d[:].rearrange("(p g) d -> p (g d)", p=P)
            )
            # per-element shift amounts 0..15 on every partition (runs during
            # the obs DMA latency window)
            nc.gpsimd.iota(sh_t[:], pattern=[[1, D]], channel_multiplier=0)

            with nc.allow_low_precision(reason="exact small-int bit packing"):
                for j in range(G):
                    # idx = sum_d obs[:,d] << d
                    nc.vector.tensor_tensor(
                        out=prod_t[:],
                        in0=obs_t[:, j * D : (j + 1) * D],
                        in1=sh_t[:],
                        op=mybir.AluOpType.logical_shift_left,
                    )
                    nc.vector.tensor_reduce(
                        out=idx_t[:, j : j + 1],
                        in_=prod_t[:],
                        axis=mybir.AxisListType.X,
                        op=mybir.AluOpType.add,
                    )
                    # HW indirect DMA: ONE offset per partition; fetches the
                    # out free-dim length from that offset. 128 rows per shot.
                    nc.gpsimd.indirect_dma_start(
                        out=g_t[:, j : j + 1],
                        out_offset=None,
                        in_=v_d[:],
                        in_offset=bass.IndirectOffsetOnAxis(
                            ap=idx_t[:, j : j + 1], axis=0
                        ),
                        oob_is_err=False,
                    )
            # single store: per-group 4B-grain stores contend with the
            # gathers on the SDMA rings and push later gather receipts out
            # (~3.7us regression measured), so write everything once
            nc.sync.dma_start(
                out=out_d[:].rearrange("(p g) -> p g", p=P), in_=g_t[:]
            )

    nc.compile()
    _CACHE["nc"] = nc
    return nc


def _fold_table(mask: np.ndarray, v: np.ndarray) -> np.ndarray:
    """Fold (mask, v) into table[key] = sum of v[n] over rows n with that key."""
    pw = 1 << np.arange(D, dtype=np.int64)
    m01 = (np.asarray(mask).astype(np.int64) + 1) // 2
    keys = (m01 * pw[None, :]).sum(axis=1)
    if np.array_equal(keys, np.arange(N, dtype=np.int64)):
        return v  # the actual setup: mask enumerates all states in order
    table = np.zeros(N, dtype=np.float32)
    np.add.at(table, keys, v)
    return table


def kernel(obs, mask, v):
    global LAST_RESULT
    from concourse.bass_utils import run_bass_kernel_spmd

    obs = np.ascontiguousarray(np.asarray(obs), dtype=np.int32)
    v = np.ascontiguousarray(np.asarray(v), dtype=np.float32)
    table = np.ascontiguousarray(_fold_table(mask, v)).reshape(N, 1)

    nc = _program()
    in_maps = [
        {"obs": obs[i * BS : (i + 1) * BS], "v": table} for i in range(N_CORES)
    ]
    res = run_bass_kernel_spmd(nc, in_maps, list(range(N_CORES)))
    LAST_RESULT = res
    return np.concatenate(
        [res.results[i]["out"].reshape(BS) for i in range(N_CORES)]
    )


# revision 9
# speedup vs baseline: 1.2370x; 1.1305x over previous
"""Trainium2 Bass kernel for nn_CriticTab (embedding-lookup critic table).

Reference math: out[b] = sum_n [obs[b,:] exactly matches bits of mask row n] * v[n].
Since the match indicator keys on the packed bit-pattern of each mask row, the
(mask, v) pair folds on the host into a 65536-entry table indexed by packed obs
bits (exact for ANY {-1,+1} mask, including permuted/duplicate/missing rows —
collisions sum, missing keys give 0, which is precisely the reference's
multi-/zero-hot `ind @ v`). The device kernel packs obs bits into an index
(DVE shift-by-iota + reduce-add) and gathers from the table in HBM via SWDGE
indirect DMA (one offset per partition per gather → 4 gathers of 128).
Data-parallel over B across the 8 cores.
"""

import numpy as np

B, D, N = 4096, 16, 65536
N_CORES = 8
BS = B // N_CORES  # 512 observations per core
P = 128            # SBUF partitions
G = BS // P        # 4 groups of 128 observations

_CACHE = {}

LAST_RESULT = None  # BassKernelResults of the most recent run (for test harness)


def _program():
    """Build + compile the single-core SPMD Bass program (cached per process)."""
    if "nc" in _CACHE:
        return _CACHE["nc"]

    import concourse.bacc as bacc
    import concourse.bass as bass
    import concourse.mybir as mybir
    import concourse.tile as tile

    nc = bacc.Bacc("TRN2", debug=False, enable_asserts=False, num_devices=N_CORES)
    obs_d = nc.dram_tensor("obs", [BS, D], mybir.dt.int32, kind="ExternalInput")
    v_d = nc.dram_tensor("v", [N, 1], mybir.dt.float32, kind="ExternalInput")
    out_d = nc.dram_tensor("out", [BS], mybir.dt.float32, kind="ExternalOutput")

    with tile.TileContext(nc) as tc:
        with tc.tile_pool(name="p", bufs=1) as pool:
            obs_t = pool.tile([P, G * D], mybir.dt.int32)
            sh_t = pool.tile([P, G * D], mybir.dt.int32)
            prod_t = pool.tile([P, G * D], mybir.dt.int32)
            idx_t = pool.tile([P, G], mybir.dt.int32)
            g_t = pool.tile([P, G], mybir.dt.float32)

            # obs rows p*G+g land on partition p, giving each partition one
            # contiguous 256B run of DRAM.
            nc.sync.dma_start(
                out=obs_t[:], in_=obs_d[:].rearrange("(p g) d -> p (g d)", p=P)
            )
            # per-element shift amounts 0..15 repeated per group (runs during
            # the obs DMA latency window)
            nc.gpsimd.iota(sh_t[:], pattern=[[0, G], [1, D]], channel_multiplier=0)

            with nc.allow_low_precision(reason="exact small-int bit packing"):
                # idx = sum_d obs[:,d] << d, all 4 groups in two DVE ops so
                # every index is ready before the first gather dispatches
                nc.vector.tensor_tensor(
                    out=prod_t[:],
                    in0=obs_t[:],
                    in1=sh_t[:],
                    op=mybir.AluOpType.logical_shift_left,
                )
                nc.vector.tensor_reduce(
                    out=idx_t[:],
                    in_=prod_t[:].rearrange("p (g d) -> p g d", d=D),
                    axis=mybir.AxisListType.X,
                    op=mybir.AluOpType.add,
                )
            for j in range(G):
                # HW indirect DMA: ONE offset per partition; fetches the
                # out free-dim length from that offset. 128 rows per shot.
                nc.gpsimd.indirect_dma_start(
                    out=g_t[:, j : j + 1],
                    out_offset=None,
                    in_=v_d[:],
                    in_offset=bass.IndirectOffsetOnAxis(
                        ap=idx_t[:, j : j + 1], axis=0
                    ),
                    oob_is_err=False,
                )
            # single store: per-group 4B-grain stores contend with the
            # gathers on the SDMA rings and push later gather receipts out
            # (~3.7us regression measured), so write everything once
            nc.sync.dma_start(
                out=out_d[:].rearrange("(p g) -> p g", p=P), in_=g_t[:]
            )

    nc.compile()
    _CACHE["nc"] = nc
    return nc


def _fold_table(mask: np.ndarray, v: np.ndarray) -> np.ndarray:
    """Fold (mask, v) into table[key] = sum of v[n] over rows n with that key."""
    pw = 1 << np.arange(D, dtype=np.int64)
    m01 = (np.asarray(mask).astype(np.int64) + 1) // 2
    keys = (m01 * pw[None, :]).sum(axis=1)
    if np.array_equal(keys, np.arange(N, dtype=np.int64)):
        return v  # the actual setup: mask enumerates all states in order
    table = np.zeros(N, dtype=np.float32)
    np.add.at(table, keys, v)
    return table


def kernel(obs, mask, v):
    global LAST_RESULT
    from concourse.bass_utils import run_bass_kernel_spmd

    obs = np.ascontiguousarray(np.asarray(obs), dtype=np.int32)
    v = np.ascontiguousarray(np.asarray(v), dtype=np.float32)
    table = np.ascontiguousarray(_fold_table(mask, v)).reshape(N, 1)

    nc = _program()
    in_maps = [
        {"obs": obs[i * BS : (i + 1) * BS], "v": table} for i in range(N_CORES)
    ]
    res = run_bass_kernel_spmd(nc, in_maps, list(range(N_CORES)))
    LAST_RESULT = res
    return np.concatenate(
        [res.results[i]["out"].reshape(BS) for i in range(N_CORES)]
    )
